# revision 31
# baseline (speedup 1.0000x reference)
"""TopK sparse autoencoder forward pass on 8 TRN2 NeuronCores.

Data-parallel over the batch: each core owns 512 rows and runs an identical
program (SPMD).  Per core:

  A. encode:  acts = relu((x - b_dec) @ W_enc.T + b_enc)
     - single-pass fp32r matmul (PE "replicated fp32" mode: ~1.4x bf16 cost,
       rms err ~8e-5 vs exact — measured on HW), vs 3x for a bf16 hi/lo
       split and 4x for native fp32
     - W_enc streamed from HBM once (f32); fp32 acts spilled to DRAM
     - per-256-chunk top-8 candidates extracted from drain bounces (DVE max8)
  B. topk:    exact top-k threshold tau from the candidate array via
     iterated max8 + match_replace; exactness flag per row
  C. mask:    enc = (acts >= tau) * acts  (f32 compare, bf16 result),
     DMA-xbar-transposed to [F, B]; transposes are dispatched from the
     Activation queue so they never head-of-line-block the streaming DMAs
  D. decode:  x_hat = enc @ W_dec.T + b_dec   [bf16, encoded-stationary]
     - the whole C/D pipeline is emitted reload->mask->transpose ahead of
       the matmul stream; pool depths give ~2-3 blocks of lookahead so the
       PE never waits on the xbar

The per-chunk top-8 candidate set provably contains the true top-k unless
some 256-wide chunk holds >8 of the top-k values; that condition is detected
on-device (flag = chunk-8th-largest > tau) and the handful of flagged rows
(expected: zero) are recomputed exactly on the host.
"""

import numpy as np
import ml_dtypes

ACT_DIM = 768
DICT = 24576
BATCH = 4096
NCORES = 8
ROWS = BATCH // NCORES          # 512 rows per core
NT = ROWS // 128                # 4 row-tiles per core
CH = 256                        # stage-1 chunk width
NCH = DICT // CH                # 96 chunks
CANDW = NCH * 8                 # 768 candidates per row
NEG = -1.0e30
BF16 = ml_dtypes.bfloat16
NA = ACT_DIM // 128             # 6 K-chunks

_cache = {}


def _build(k: int, with_benc: bool):
    import concourse.bass as bass
    import concourse.mybir as mybir
    from concourse import bacc
    from concourse import tile

    f32 = mybir.dt.float32
    f32r = mybir.dt.float32r
    bf16 = mybir.dt.bfloat16
    ROUNDS = (k + 7) // 8

    nc = bacc.Bacc("TRN2", target_bir_lowering=False, debug=False,
                   num_devices=NCORES)

    xt_d = nc.dram_tensor("xt", [ACT_DIM, ROWS], f32r, kind="ExternalInput")
    wenc_d = nc.dram_tensor("wenc", [ACT_DIM, DICT], f32r,
                            kind="ExternalInput")
    wdecT_d = nc.dram_tensor("wdecT", [DICT // 1024, 128, 8 * ACT_DIM], bf16,
                             kind="ExternalInput")
    bdec_d = nc.dram_tensor("bdec", [1, ACT_DIM], f32, kind="ExternalInput")
    ident_d = nc.dram_tensor("ident", [128, 128], bf16, kind="ExternalInput")
    if with_benc:
        benc_d = nc.dram_tensor("benc", [1, DICT], f32, kind="ExternalInput")
    xhat_d = nc.dram_tensor("xhat", [ROWS, ACT_DIM], f32, kind="ExternalOutput")
    flags_d = nc.dram_tensor("flags", [128, NT], f32, kind="ExternalOutput")
    acts_spill = nc.dram_tensor("acts_spill", [NT, 128, DICT], f32)

    NSC = DICT // 512           # 48 encode column-chunks
    NBLK = DICT // 2048         # 12 C/D blocks
    NF = DICT // 128            # 192 decoder f-chunks

    with tile.TileContext(nc) as tc:
        with tc.tile_pool(name="const", bufs=1) as constp, \
             tc.tile_pool(name="small", bufs=4 * NT + 4) as smallp, \
             tc.tile_pool(name="ac0", bufs=NT) as ac0p, \
             tc.tile_pool(name="wd0", bufs=2) as wd0p:

            bdec_row = constp.tile([1, ACT_DIM], f32)
            nc.sync.dma_start(bdec_row[:], bdec_d.ap())
            bdec_bc = constp.tile([128, ACT_DIM], f32)
            nc.gpsimd.partition_broadcast(bdec_bc[:], bdec_row[:])
            ident = constp.tile([128, 128], bf16)
            nc.sync.dma_start(ident[:], ident_d.ap())
            if with_benc:
                benc_row = constp.tile([1, DICT], f32)
                nc.sync.dma_start(benc_row[:], benc_d.ap())

            flags_sb = constp.tile([128, NT], f32)
            taus = [smallp.tile([128, 1], f32, tag="tau", name=f"tau{t}")
                    for t in range(NT)]

            def topk(t):
                # exact tau per row-tile from the candidate array
                c8 = smallp.tile([128, 1], f32, tag="c8", name=f"c8_{t}")
                cand3 = cands[t][:].rearrange("p (c e) -> p c e", e=8)
                nc.vector.tensor_reduce(c8[:], cand3[:, :, 7:8],
                                        axis=mybir.AxisListType.XY,
                                        op=mybir.AluOpType.max)
                topv = smallp.tile([128, 8 * ROUNDS], f32, tag="topv",
                                   name=f"topv{t}")
                for r in range(ROUNDS):
                    nc.vector.max(topv[:, r * 8:(r + 1) * 8], cands[t][:])
                    if r < ROUNDS - 1:
                        nc.vector.match_replace(
                            cands[t][:], topv[:, r * 8:(r + 1) * 8],
                            cands[t][:], NEG)
                nc.vector.tensor_copy(taus[t][:], topv[:, k - 1:k])
                nc.vector.tensor_tensor(flags_sb[:, t:t + 1], c8[:],
                                        taus[t][:],
                                        op=mybir.AluOpType.is_gt)

            # reload/W_dec loaders; block 0 uses dedicated outer-scope pools
            # so it can prefetch during phase A
            acs = {}
            wds = {}
            pools = {"ac": None, "wd": None}        # set in the C/D scope

            def load_ac(t, blk):
                pool = ac0p if blk == 0 else pools["ac"]
                ac = pool.tile([128, 2048], f32, tag="ac",
                               name=f"ac{t}_{blk}")
                nc.sync.dma_start(
                    ac[:],
                    acts_spill.ap()[t, :, blk * 2048:(blk + 1) * 2048])
                acs[(t, blk)] = ac

            def load_wd(blk):
                for g in range(2):
                    pool = wd0p if blk == 0 else pools["wd"]
                    wd = pool.tile([128, 8, ACT_DIM], bf16, tag="wd",
                                   name=f"wd{blk}_{g}")
                    nc.sync.dma_start(
                        wd[:].rearrange("p c a -> p (c a)"),
                        wdecT_d.ap()[blk * 2 + g, :, :])
                    wds[(blk, g)] = wd

            # ---------------- Phase A: encode + spill + stage-1 ----------
            with tc.tile_pool(name="xt", bufs=1) as xtp, \
                 tc.tile_pool(name="cand", bufs=NT) as candp, \
                 tc.tile_pool(name="wenc", bufs=5) as wencp, \
                 tc.tile_pool(name="bounce", bufs=6) as bouncep, \
                 tc.tile_pool(name="encpsum", bufs=8, space="PSUM") as encpsp, \
                 tc.tile_pool(name="bencbc", bufs=2) as bencbcp:

                cands = [candp.tile([128, CANDW], f32, tag="cand",
                                    name=f"cand{t}") for t in range(NT)]
                x_sb = xtp.tile([128, NA, ROWS], f32r)
                nc.sync.dma_start(
                    x_sb[:], xt_d.ap().rearrange("(a p) r -> p a r", p=128))

                for sc in range(NSC):
                    if sc == NSC // 2:
                        # phase-C prefetch: blk 0 acts + first W_dec chunk
                        # (their spill chunks are long since written)
                        for t in range(NT):
                            load_ac(t, 0)
                        load_wd(0)
                    wch = wencp.tile([128, NA, 512], f32r, tag="w",
                                     name=f"w{sc}")
                    nc.sync.dma_start(
                        wch[:],
                        wenc_d.ap()[:, sc * 512:(sc + 1) * 512]
                        .rearrange("(a p) c -> p a c", p=128))
                    if with_benc:
                        bb = bencbcp.tile([128, 512], f32, tag="bb")
                        nc.gpsimd.partition_broadcast(
                            bb[:], benc_row[0:1, sc * 512:(sc + 1) * 512])
                    for t in range(NT):
                        ps = encpsp.tile([128, 512], f32, tag="eps")
                        rt = slice(t * 128, (t + 1) * 128)
                        for a in range(NA):
                            nc.tensor.matmul(
                                ps[:], x_sb[:, a, rt], wch[:, a, :],
                                start=(a == 0), stop=(a == NA - 1))
                        bo = bouncep.tile([128, 512], f32, tag="bo")
                        if with_benc:
                            nc.vector.tensor_tensor(bo[:], ps[:], bb[:],
                                                    op=mybir.AluOpType.add)
                            nc.scalar.activation(
                                bo[:], bo[:], mybir.ActivationFunctionType.Relu)
                        else:
                            nc.scalar.activation(
                                bo[:], ps[:], mybir.ActivationFunctionType.Relu)
                        nc.sync.dma_start(
                            acts_spill.ap()[t, :, sc * 512:(sc + 1) * 512], bo[:])
                        for cc in range(512 // CH):
                            c = sc * (512 // CH) + cc
                            nc.vector.max(
                                cands[t][:, c * 8:(c + 1) * 8],
                                bo[:, cc * CH:(cc + 1) * CH])
                        if sc == NSC - 1:
                            # tau extraction overlaps the remaining tiles'
                            # last-chunk matmuls on the PE
                            topk(t)

            # -------- Phases B+C+D: threshold, mask/transpose, decode ----
            with tc.tile_pool(name="actsc", bufs=4) as actscp, \
                 tc.tile_pool(name="encb", bufs=3) as encbp, \
                 tc.tile_pool(name="enct", bufs=8) as enctp, \
                 tc.tile_pool(name="wdec", bufs=4) as wdecp, \
                 tc.tile_pool(name="decps_hi", bufs=1, space="PSUM") as dphp, \
                 tc.tile_pool(name="decps_lo", bufs=1, space="PSUM") as dplp, \
                 tc.tile_pool(name="tpsum", bufs=2, space="PSUM") as tpp, \
                 tc.tile_pool(name="outsb", bufs=2) as outp:

                pools["ac"] = actscp
                pools["wd"] = wdecp
                ets = {}
                # decode accumulators, packed bank-aligned: [0:512] parts in
                # 4 full banks, [512:768] parts packed 2-per-bank.  The
                # packed lo chains must never issue start=True: the PSUM
                # zero-region of a start is bank-granular and would clobber
                # the neighbor chain.  memset once, accumulate start=False.
                dph = dphp.tile([128, NT, 512], f32)
                dpl = dplp.tile([128, NT, 256], f32)
                nc.vector.memset(dpl[:], 0.0)

                def mask_transpose(t, blk, ac):
                    # mask in f32 (exact selection) -> eb bf16; transpose on
                    # the PE (identity matmul) via PSUM, DVE drains to SBUF
                    eb = encbp.tile([128, 2048], bf16, tag="eb",
                                    name=f"eb{t}_{blk}")
                    nc.vector.scalar_tensor_tensor(
                        eb[:], ac[:], taus[t][:, 0:1], ac[:],
                        op0=mybir.AluOpType.is_ge,
                        op1=mybir.AluOpType.mult)
                    et = enctp.tile([128, 16, 128], bf16, tag="enct",
                                    name=f"et{t}_{blk}")
                    for h in range(2):
                        tps = tpp.tile([128, 8, 128], bf16, tag="tps")
                        for j in range(8):
                            c = h * 8 + j
                            nc.tensor.transpose(
                                tps[:, j, :],
                                eb[:, c * 128:(c + 1) * 128], ident[:])
                        nc.vector.tensor_copy(
                            et[:, h * 8:(h + 1) * 8, :], tps[:])
                    ets[(t, blk)] = et

                def decode(blk):
                    for g in range(2):
                        wd = wds.pop((blk, g))
                        for t in range(NT):
                            for j in range(8):
                                f = blk * 16 + g * 8 + j
                                lhsT = ets[(t, blk)][:, g * 8 + j, :]
                                st = (f == 0)
                                sp = (f == NF - 1)
                                nc.tensor.matmul(
                                    dph[:, t, :], lhsT, wd[:, j, 0:512],
                                    start=st, stop=sp)
                                nc.tensor.matmul(
                                    dpl[:, t, :], lhsT,
                                    wd[:, j, 512:ACT_DIM],
                                    start=False, stop=sp)
                    for t in range(NT):
                        ets.pop((t, blk))

                # taus were computed at the tail of phase A; blk 0 acts and
                # W_dec chunk 0 were prefetched mid-phase-A.  The PE can mask
                # and transpose blk 0 immediately.
                for t in range(NT):
                    load_ac(t, 1)
                    mask_transpose(t, 0, acs.pop((t, 0)))

                for blk in range(1, NBLK):
                    load_wd(blk)
                    for t in range(NT):
                        if blk + 1 < NBLK:
                            load_ac(t, blk + 1)
                        mask_transpose(t, blk, acs.pop((t, blk)))
                    decode(blk - 1)
                decode(NBLK - 1)
                for t in range(NT):
                    ot = outp.tile([128, ACT_DIM], f32, tag="ot",
                                   name=f"ot{t}")
                    nc.vector.tensor_tensor(ot[:, 0:512], dph[:, t, :],
                                            bdec_bc[:, 0:512],
                                            op=mybir.AluOpType.add)
                    nc.vector.tensor_tensor(ot[:, 512:ACT_DIM], dpl[:, t, :],
                                            bdec_bc[:, 512:ACT_DIM],
                                            op=mybir.AluOpType.add)
                    nc.sync.dma_start(
                        xhat_d.ap()[t * 128:(t + 1) * 128, :], ot[:])
                nc.sync.dma_start(flags_d.ap(), flags_sb[:])

    nc.compile()
    return nc


def _get_program(k: int, with_benc: bool):
    key = (k, with_benc)
    if key not in _cache:
        _cache[key] = _build(k, with_benc)
    return _cache[key]


def _host_repair(out, rows, x, W_enc, b_enc, W_dec, b_dec, k):
    for r in rows:
        pre = (x[r] - b_dec) @ W_enc.T + b_enc
        acts = np.maximum(pre, 0.0)
        idx = np.argsort(-acts, kind="stable")[:k]
        enc = np.zeros_like(acts)
        enc[idx] = acts[idx]
        out[r] = enc @ W_dec.T + b_dec


def run(inputs, trace=False):
    from concourse.bass_utils import run_bass_kernel_spmd

    x = np.asarray(inputs["x"], dtype=np.float32)
    W_enc = np.asarray(inputs["W_enc"], dtype=np.float32)
    b_enc = np.asarray(inputs["b_enc"], dtype=np.float32)
    W_dec = np.asarray(inputs["W_dec"], dtype=np.float32)
    b_dec = np.asarray(inputs["b_dec"], dtype=np.float32)
    k = int(np.asarray(inputs["k"]))
    assert x.shape == (BATCH, ACT_DIM) and W_enc.shape == (DICT, ACT_DIM)
    assert 1 <= k <= CANDW - 8

    with_benc = bool(np.any(b_enc))
    nc = _get_program(k, with_benc)

    xT = np.ascontiguousarray((x - b_dec).T, dtype=np.float32)
    wencT = np.ascontiguousarray(W_enc.T, dtype=np.float32)
    wdecT = np.ascontiguousarray(W_dec.T).astype(BF16)
    # [NFG, 128, 8*ACT_DIM]: partition p of group fg holds rows of the 8
    # 128-row f-chunks, giving 12KB contiguous per-partition DMA reads
    wdec_r = np.ascontiguousarray(
        wdecT.reshape(DICT // 1024, 8, 128, ACT_DIM).transpose(0, 2, 1, 3)
        .reshape(DICT // 1024, 128, 8 * ACT_DIM))
    bdec_row = np.ascontiguousarray(b_dec.reshape(1, ACT_DIM))

    in_maps = []
    for c in range(NCORES):
        sl = slice(c * ROWS, (c + 1) * ROWS)
        m = {
            "xt": np.ascontiguousarray(xT[:, sl]),
            "wenc": wencT,
            "wdecT": wdec_r,
            "bdec": bdec_row,
            "ident": np.eye(128, dtype=BF16),
        }
        if with_benc:
            m["benc"] = np.ascontiguousarray(b_enc.reshape(1, DICT))
        in_maps.append(m)

    res = run_bass_kernel_spmd(nc, in_maps, core_ids=list(range(NCORES)),
                               trace=trace)

    out = np.empty((BATCH, ACT_DIM), dtype=np.float32)
    flagged = []
    for c in range(NCORES):
        out[c * ROWS:(c + 1) * ROWS] = res.results[c]["xhat"]
        fl = res.results[c]["flags"]          # [128, NT]
        for t in range(NT):
            for p in np.nonzero(fl[:, t] > 0)[0]:
                flagged.append(c * ROWS + t * 128 + int(p))
    if flagged:
        _host_repair(out, flagged, x, W_enc, b_enc, W_dec, b_dec, k)
    return out, res, flagged


def kernel(**inputs) -> np.ndarray:
    out, _, _ = run(inputs)
    return out


# revision 33
# speedup vs baseline: 1.0238x; 1.0238x over previous
"""TopK sparse autoencoder forward pass on 8 TRN2 NeuronCores.

Data-parallel over the batch: each core owns 512 rows and runs an identical
program (SPMD).  Per core:

  A. encode:  acts = relu((x - b_dec) @ W_enc.T + b_enc)
     - single-pass fp32r matmul (PE "replicated fp32" mode: ~1.4x bf16 cost,
       rms err ~8e-5 vs exact — measured on HW), vs 3x for a bf16 hi/lo
       split and 4x for native fp32
     - W_enc streamed from HBM once (f32); fp32 acts spilled to DRAM
     - per-256-chunk top-8 candidates extracted from drain bounces (DVE max8)
  B. topk:    exact top-k threshold tau from the candidate array via
     iterated max8 + match_replace; exactness flag per row
  C. mask:    enc = (acts >= tau) * acts  (f32 compare, bf16 result),
     DMA-xbar-transposed to [F, B]; transposes are dispatched from the
     Activation queue so they never head-of-line-block the streaming DMAs
  D. decode:  x_hat = enc @ W_dec.T + b_dec   [bf16, encoded-stationary]
     - the whole C/D pipeline is emitted reload->mask->transpose ahead of
       the matmul stream; pool depths give ~2-3 blocks of lookahead so the
       PE never waits on the xbar

The per-chunk top-8 candidate set provably contains the true top-k unless
some 256-wide chunk holds >8 of the top-k values; that condition is detected
on-device (flag = chunk-8th-largest > tau) and the handful of flagged rows
(expected: zero) are recomputed exactly on the host.
"""

import numpy as np
import ml_dtypes

ACT_DIM = 768
DICT = 24576
BATCH = 4096
NCORES = 8
ROWS = BATCH // NCORES          # 512 rows per core
NT = ROWS // 128                # 4 row-tiles per core
CH = 256                        # stage-1 chunk width
NCH = DICT // CH                # 96 chunks
CANDW = NCH * 8                 # 768 candidates per row
NEG = -1.0e30
BF16 = ml_dtypes.bfloat16
NA = ACT_DIM // 128             # 6 K-chunks

_cache = {}


def _build(k: int, with_benc: bool):
    import concourse.bass as bass
    import concourse.mybir as mybir
    from concourse import bacc
    from concourse import tile

    f32 = mybir.dt.float32
    f32r = mybir.dt.float32r
    bf16 = mybir.dt.bfloat16
    ROUNDS = (k + 7) // 8

    nc = bacc.Bacc("TRN2", target_bir_lowering=False, debug=False,
                   num_devices=NCORES)

    xt_d = nc.dram_tensor("xt", [ACT_DIM, ROWS], f32r, kind="ExternalInput")
    wenc_d = nc.dram_tensor("wenc", [ACT_DIM, DICT], f32r,
                            kind="ExternalInput")
    wdecT_d = nc.dram_tensor("wdecT", [DICT // 1024, 128, 8 * ACT_DIM], bf16,
                             kind="ExternalInput")
    bdec_d = nc.dram_tensor("bdec", [1, ACT_DIM], f32, kind="ExternalInput")
    ident_d = nc.dram_tensor("ident", [128, 128], bf16, kind="ExternalInput")
    if with_benc:
        benc_d = nc.dram_tensor("benc", [1, DICT], f32, kind="ExternalInput")
    xhat_d = nc.dram_tensor("xhat", [ROWS, ACT_DIM], f32, kind="ExternalOutput")
    flags_d = nc.dram_tensor("flags", [128, NT], f32, kind="ExternalOutput")
    acts_spill = nc.dram_tensor("acts_spill", [NT, 128, DICT], f32)

    NSC = DICT // 512           # 48 encode column-chunks
    NBLK = DICT // 2048         # 12 C/D blocks
    NF = DICT // 128            # 192 decoder f-chunks

    with tile.TileContext(nc) as tc:
        with tc.tile_pool(name="const", bufs=1) as constp, \
             tc.tile_pool(name="small", bufs=4 * NT + 4) as smallp, \
             tc.tile_pool(name="ac0", bufs=NT) as ac0p, \
             tc.tile_pool(name="wd0", bufs=2) as wd0p:

            bdec_row = constp.tile([1, ACT_DIM], f32)
            nc.sync.dma_start(bdec_row[:], bdec_d.ap())
            bdec_bc = constp.tile([128, ACT_DIM], f32)
            nc.gpsimd.partition_broadcast(bdec_bc[:], bdec_row[:])
            ident = constp.tile([128, 128], bf16)
            nc.sync.dma_start(ident[:], ident_d.ap())
            if with_benc:
                benc_row = constp.tile([1, DICT], f32)
                nc.sync.dma_start(benc_row[:], benc_d.ap())

            flags_sb = constp.tile([128, NT], f32)
            taus = [smallp.tile([128, 1], f32, tag="tau", name=f"tau{t}")
                    for t in range(NT)]

            def topk(t):
                # exact tau per row-tile from the candidate array
                c8 = smallp.tile([128, 1], f32, tag="c8", name=f"c8_{t}")
                cand3 = cands[t][:].rearrange("p (c e) -> p c e", e=8)
                nc.vector.tensor_reduce(c8[:], cand3[:, :, 7:8],
                                        axis=mybir.AxisListType.XY,
                                        op=mybir.AluOpType.max)
                topv = smallp.tile([128, 8 * ROUNDS], f32, tag="topv",
                                   name=f"topv{t}")
                for r in range(ROUNDS):
                    nc.vector.max(topv[:, r * 8:(r + 1) * 8], cands[t][:])
                    if r < ROUNDS - 1:
                        nc.vector.match_replace(
                            cands[t][:], topv[:, r * 8:(r + 1) * 8],
                            cands[t][:], NEG)
                nc.vector.tensor_copy(taus[t][:], topv[:, k - 1:k])
                nc.vector.tensor_tensor(flags_sb[:, t:t + 1], c8[:],
                                        taus[t][:],
                                        op=mybir.AluOpType.is_gt)

            # reload/W_dec loaders; block 0 uses dedicated outer-scope pools
            # so it can prefetch during phase A
            acs = {}
            wds = {}
            pools = {"ac": None, "wd": None}        # set in the C/D scope

            def load_ac(t, blk, eng=None):
                pool = ac0p if blk == 0 else pools["ac"]
                ac = pool.tile([128, 2048], f32, tag="ac",
                               name=f"ac{t}_{blk}")
                (eng or nc.sync).dma_start(
                    ac[:],
                    acts_spill.ap()[t, :, blk * 2048:(blk + 1) * 2048])
                acs[(t, blk)] = ac

            def load_wd(blk, eng=None):
                for g in range(2):
                    pool = wd0p if blk == 0 else pools["wd"]
                    wd = pool.tile([128, 8, ACT_DIM], bf16, tag="wd",
                                   name=f"wd{blk}_{g}")
                    (eng or nc.sync).dma_start(
                        wd[:].rearrange("p c a -> p (c a)"),
                        wdecT_d.ap()[blk * 2 + g, :, :])
                    wds[(blk, g)] = wd

            # ---------------- Phase A: encode + spill + stage-1 ----------
            with tc.tile_pool(name="xt", bufs=1) as xtp, \
                 tc.tile_pool(name="cand", bufs=NT) as candp, \
                 tc.tile_pool(name="wenc", bufs=5) as wencp, \
                 tc.tile_pool(name="bounce", bufs=6) as bouncep, \
                 tc.tile_pool(name="encpsum", bufs=8, space="PSUM") as encpsp, \
                 tc.tile_pool(name="bencbc", bufs=2) as bencbcp:

                cands = [candp.tile([128, CANDW], f32, tag="cand",
                                    name=f"cand{t}") for t in range(NT)]
                x_sb = xtp.tile([128, NA, ROWS], f32r)
                nc.sync.dma_start(
                    x_sb[:], xt_d.ap().rearrange("(a p) r -> p a r", p=128))

                def load_w(sc):
                    wch = wencp.tile([128, NA, 512], f32r, tag="w",
                                     name=f"w{sc}")
                    nc.sync.dma_start(
                        wch[:],
                        wenc_d.ap()[:, sc * 512:(sc + 1) * 512]
                        .rearrange("(a p) c -> p a c", p=128))
                    if with_benc:
                        bb = bencbcp.tile([128, 512], f32, tag="bb")
                        nc.gpsimd.partition_broadcast(
                            bb[:], benc_row[0:1, sc * 512:(sc + 1) * 512])
                    else:
                        bb = None
                    return wch, bb

                def enc_chunk(sc, t, wch, bb):
                    ps = encpsp.tile([128, 512], f32, tag="eps")
                    rt = slice(t * 128, (t + 1) * 128)
                    for a in range(NA):
                        nc.tensor.matmul(
                            ps[:], x_sb[:, a, rt], wch[:, a, :],
                            start=(a == 0), stop=(a == NA - 1))
                    bo = bouncep.tile([128, 512], f32, tag="bo")
                    if with_benc:
                        nc.vector.tensor_tensor(bo[:], ps[:], bb[:],
                                                op=mybir.AluOpType.add)
                        nc.scalar.activation(
                            bo[:], bo[:], mybir.ActivationFunctionType.Relu)
                    else:
                        nc.scalar.activation(
                            bo[:], ps[:], mybir.ActivationFunctionType.Relu)
                    nc.sync.dma_start(
                        acts_spill.ap()[t, :, sc * 512:(sc + 1) * 512], bo[:])
                    for cc in range(512 // CH):
                        c = sc * (512 // CH) + cc
                        nc.vector.max(
                            cands[t][:, c * 8:(c + 1) * 8],
                            bo[:, cc * CH:(cc + 1) * CH])

                TAIL = 4
                for sc in range(NSC - TAIL):
                    if sc == NSC // 2:
                        # phase-C prefetch: blk 0 acts + first W_dec chunk
                        # (spilled long ago); dispatched from the Activation
                        # queue so the W-enc stream is undisturbed
                        for t in range(NT):
                            load_ac(t, 0, eng=nc.scalar)
                        load_wd(0, eng=nc.scalar)
                    wch, bb = load_w(sc)
                    for t in range(NT):
                        enc_chunk(sc, t, wch, bb)
                # tail: tile-major so each tile's tau extraction overlaps the
                # remaining tiles' encode matmuls on the PE
                tail_w = {sc: load_w(sc) for sc in range(NSC - TAIL, NSC)}
                for t in range(NT):
                    for sc in range(NSC - TAIL, NSC):
                        enc_chunk(sc, t, *tail_w[sc])
                    topk(t)

            # -------- Phases B+C+D: threshold, mask/transpose, decode ----
            with tc.tile_pool(name="actsc", bufs=4) as actscp, \
                 tc.tile_pool(name="encb", bufs=3) as encbp, \
                 tc.tile_pool(name="enct", bufs=8) as enctp, \
                 tc.tile_pool(name="wdec", bufs=4) as wdecp, \
                 tc.tile_pool(name="decps_hi", bufs=1, space="PSUM") as dphp, \
                 tc.tile_pool(name="decps_lo", bufs=1, space="PSUM") as dplp, \
                 tc.tile_pool(name="tpsum", bufs=2, space="PSUM") as tpp, \
                 tc.tile_pool(name="outsb", bufs=2) as outp:

                pools["ac"] = actscp
                pools["wd"] = wdecp
                ets = {}
                # decode accumulators, packed bank-aligned: [0:512] parts in
                # 4 full banks, [512:768] parts packed 2-per-bank.  The
                # packed lo chains must never issue start=True: the PSUM
                # zero-region of a start is bank-granular and would clobber
                # the neighbor chain.  memset once, accumulate start=False.
                dph = dphp.tile([128, NT, 512], f32)
                dpl = dplp.tile([128, NT, 256], f32)
                nc.vector.memset(dpl[:], 0.0)

                def mask_transpose(t, blk, ac):
                    # mask in f32 (exact selection) -> eb bf16; transpose on
                    # the PE (identity matmul) via PSUM, DVE drains to SBUF
                    eb = encbp.tile([128, 2048], bf16, tag="eb",
                                    name=f"eb{t}_{blk}")
                    nc.vector.scalar_tensor_tensor(
                        eb[:], ac[:], taus[t][:, 0:1], ac[:],
                        op0=mybir.AluOpType.is_ge,
                        op1=mybir.AluOpType.mult)
                    et = enctp.tile([128, 16, 128], bf16, tag="enct",
                                    name=f"et{t}_{blk}")
                    for h in range(2):
                        tps = tpp.tile([128, 8, 128], bf16, tag="tps")
                        for j in range(8):
                            c = h * 8 + j
                            nc.tensor.transpose(
                                tps[:, j, :],
                                eb[:, c * 128:(c + 1) * 128], ident[:])
                        nc.vector.tensor_copy(
                            et[:, h * 8:(h + 1) * 8, :], tps[:])
                    ets[(t, blk)] = et

                def decode(blk):
                    for g in range(2):
                        wd = wds.pop((blk, g))
                        for t in range(NT):
                            for j in range(8):
                                f = blk * 16 + g * 8 + j
                                lhsT = ets[(t, blk)][:, g * 8 + j, :]
                                st = (f == 0)
                                sp = (f == NF - 1)
                                nc.tensor.matmul(
                                    dph[:, t, :], lhsT, wd[:, j, 0:512],
                                    start=st, stop=sp)
                                nc.tensor.matmul(
                                    dpl[:, t, :], lhsT,
                                    wd[:, j, 512:ACT_DIM],
                                    start=False, stop=sp)
                    for t in range(NT):
                        ets.pop((t, blk))

                # taus were computed at the tail of phase A; blk 0 acts and
                # W_dec chunk 0 were prefetched mid-phase-A.  The PE can mask
                # and transpose blk 0 immediately.
                for t in range(NT):
                    load_ac(t, 1)
                    mask_transpose(t, 0, acs.pop((t, 0)))

                for blk in range(1, NBLK):
                    load_wd(blk)
                    for t in range(NT):
                        if blk + 1 < NBLK:
                            load_ac(t, blk + 1)
                        mask_transpose(t, blk, acs.pop((t, blk)))
                    decode(blk - 1)
                decode(NBLK - 1)
                for t in range(NT):
                    ot = outp.tile([128, ACT_DIM], f32, tag="ot",
                                   name=f"ot{t}")
                    nc.vector.tensor_tensor(ot[:, 0:512], dph[:, t, :],
                                            bdec_bc[:, 0:512],
                                            op=mybir.AluOpType.add)
                    nc.vector.tensor_tensor(ot[:, 512:ACT_DIM], dpl[:, t, :],
                                            bdec_bc[:, 512:ACT_DIM],
                                            op=mybir.AluOpType.add)
                    nc.sync.dma_start(
                        xhat_d.ap()[t * 128:(t + 1) * 128, :], ot[:])
                nc.sync.dma_start(flags_d.ap(), flags_sb[:])

    nc.compile()
    return nc


def _get_program(k: int, with_benc: bool):
    key = (k, with_benc)
    if key not in _cache:
        _cache[key] = _build(k, with_benc)
    return _cache[key]


def _host_repair(out, rows, x, W_enc, b_enc, W_dec, b_dec, k):
    for r in rows:
        pre = (x[r] - b_dec) @ W_enc.T + b_enc
        acts = np.maximum(pre, 0.0)
        idx = np.argsort(-acts, kind="stable")[:k]
        enc = np.zeros_like(acts)
        enc[idx] = acts[idx]
        out[r] = enc @ W_dec.T + b_dec


def run(inputs, trace=False):
    from concourse.bass_utils import run_bass_kernel_spmd

    x = np.asarray(inputs["x"], dtype=np.float32)
    W_enc = np.asarray(inputs["W_enc"], dtype=np.float32)
    b_enc = np.asarray(inputs["b_enc"], dtype=np.float32)
    W_dec = np.asarray(inputs["W_dec"], dtype=np.float32)
    b_dec = np.asarray(inputs["b_dec"], dtype=np.float32)
    k = int(np.asarray(inputs["k"]))
    assert x.shape == (BATCH, ACT_DIM) and W_enc.shape == (DICT, ACT_DIM)
    assert 1 <= k <= CANDW - 8

    with_benc = bool(np.any(b_enc))
    nc = _get_program(k, with_benc)

    xT = np.ascontiguousarray((x - b_dec).T, dtype=np.float32)
    wencT = np.ascontiguousarray(W_enc.T, dtype=np.float32)
    wdecT = np.ascontiguousarray(W_dec.T).astype(BF16)
    # [NFG, 128, 8*ACT_DIM]: partition p of group fg holds rows of the 8
    # 128-row f-chunks, giving 12KB contiguous per-partition DMA reads
    wdec_r = np.ascontiguousarray(
        wdecT.reshape(DICT // 1024, 8, 128, ACT_DIM).transpose(0, 2, 1, 3)
        .reshape(DICT // 1024, 128, 8 * ACT_DIM))
    bdec_row = np.ascontiguousarray(b_dec.reshape(1, ACT_DIM))

    in_maps = []
    for c in range(NCORES):
        sl = slice(c * ROWS, (c + 1) * ROWS)
        m = {
            "xt": np.ascontiguousarray(xT[:, sl]),
            "wenc": wencT,
            "wdecT": wdec_r,
            "bdec": bdec_row,
            "ident": np.eye(128, dtype=BF16),
        }
        if with_benc:
            m["benc"] = np.ascontiguousarray(b_enc.reshape(1, DICT))
        in_maps.append(m)

    res = run_bass_kernel_spmd(nc, in_maps, core_ids=list(range(NCORES)),
                               trace=trace)

    out = np.empty((BATCH, ACT_DIM), dtype=np.float32)
    flagged = []
    for c in range(NCORES):
        out[c * ROWS:(c + 1) * ROWS] = res.results[c]["xhat"]
        fl = res.results[c]["flags"]          # [128, NT]
        for t in range(NT):
            for p in np.nonzero(fl[:, t] > 0)[0]:
                flagged.append(c * ROWS + t * 128 + int(p))
    if flagged:
        _host_repair(out, flagged, x, W_enc, b_enc, W_dec, b_dec, k)
    return out, res, flagged


def kernel(**inputs) -> np.ndarray:
    out, _, _ = run(inputs)
    return out


# revision 36
# speedup vs baseline: 1.0557x; 1.0312x over previous
"""TopK sparse autoencoder forward pass on 8 TRN2 NeuronCores.

Data-parallel over the batch: each core owns 512 rows and runs an identical
program (SPMD).  Per core:

  A. encode:  acts = relu((x - b_dec) @ W_enc.T + b_enc)
     - single-pass fp32r matmul (PE "replicated fp32" mode: ~1.4x bf16 cost,
       rms err ~8e-5 vs exact — measured on HW), vs 3x for a bf16 hi/lo
       split and 4x for native fp32
     - W_enc streamed from HBM once (f32); fp32 acts spilled to DRAM
     - per-256-chunk top-8 candidates extracted from drain bounces (DVE max8)
  B. topk:    exact top-k threshold tau from the candidate array via
     iterated max8 + match_replace; exactness flag per row
  C. mask:    enc = (acts >= tau) * acts  (f32 compare, bf16 result),
     DMA-xbar-transposed to [F, B]; transposes are dispatched from the
     Activation queue so they never head-of-line-block the streaming DMAs
  D. decode:  x_hat = enc @ W_dec.T + b_dec   [bf16, encoded-stationary]
     - the whole C/D pipeline is emitted reload->mask->transpose ahead of
       the matmul stream; pool depths give ~2-3 blocks of lookahead so the
       PE never waits on the xbar

The per-chunk top-8 candidate set provably contains the true top-k unless
some 256-wide chunk holds >8 of the top-k values; that condition is detected
on-device (flag = chunk-8th-largest > tau) and the handful of flagged rows
(expected: zero) are recomputed exactly on the host.
"""

import numpy as np
import ml_dtypes

ACT_DIM = 768
DICT = 24576
BATCH = 4096
NCORES = 8
ROWS = BATCH // NCORES          # 512 rows per core
NT = ROWS // 128                # 4 row-tiles per core
CH = 256                        # stage-1 chunk width
NCH = DICT // CH                # 96 chunks
CANDW = NCH * 8                 # 768 candidates per row
NEG = -1.0e30
BF16 = ml_dtypes.bfloat16
NA = ACT_DIM // 128             # 6 K-chunks

_cache = {}


def _build(k: int, with_benc: bool):
    import concourse.bass as bass
    import concourse.mybir as mybir
    from concourse import bacc
    from concourse import tile

    f32 = mybir.dt.float32
    f32r = mybir.dt.float32r
    bf16 = mybir.dt.bfloat16
    ROUNDS = (k + 7) // 8

    nc = bacc.Bacc("TRN2", target_bir_lowering=False, debug=False,
                   num_devices=NCORES)

    xt_d = nc.dram_tensor("xt", [ACT_DIM, ROWS], f32r, kind="ExternalInput")
    wenc_d = nc.dram_tensor("wenc", [ACT_DIM, DICT], f32r,
                            kind="ExternalInput")
    wdecT_d = nc.dram_tensor("wdecT", [DICT // 1024, 128, 8 * ACT_DIM], bf16,
                             kind="ExternalInput")
    bdec_d = nc.dram_tensor("bdec", [1, ACT_DIM], f32, kind="ExternalInput")
    ident_d = nc.dram_tensor("ident", [128, 128], bf16, kind="ExternalInput")
    if with_benc:
        benc_d = nc.dram_tensor("benc", [1, DICT], f32, kind="ExternalInput")
    xhat_d = nc.dram_tensor("xhat", [ROWS, ACT_DIM], f32, kind="ExternalOutput")
    flags_d = nc.dram_tensor("flags", [128, NT], f32, kind="ExternalOutput")
    acts_spill = nc.dram_tensor("acts_spill", [NT, 128, DICT], f32)

    NSC = DICT // 512           # 48 encode column-chunks
    NBLK = DICT // 2048         # 12 C/D blocks
    NF = DICT // 128            # 192 decoder f-chunks

    with tile.TileContext(nc) as tc:
        with tc.tile_pool(name="const", bufs=1) as constp, \
             tc.tile_pool(name="small", bufs=4 * NT + 4) as smallp, \
             tc.tile_pool(name="ac0", bufs=NT) as ac0p, \
             tc.tile_pool(name="wd0", bufs=2) as wd0p:

            bdec_row = constp.tile([1, ACT_DIM], f32)
            nc.sync.dma_start(bdec_row[:], bdec_d.ap())
            bdec_bc = constp.tile([128, ACT_DIM], f32)
            nc.gpsimd.partition_broadcast(bdec_bc[:], bdec_row[:])
            ident = constp.tile([128, 128], bf16)
            nc.sync.dma_start(ident[:], ident_d.ap())
            if with_benc:
                benc_row = constp.tile([1, DICT], f32)
                nc.sync.dma_start(benc_row[:], benc_d.ap())

            flags_sb = constp.tile([128, NT], f32)
            taus = [smallp.tile([128, 1], f32, tag="tau", name=f"tau{t}")
                    for t in range(NT)]

            def topk(t):
                # exact tau per row-tile from the candidate array
                c8 = smallp.tile([128, 1], f32, tag="c8", name=f"c8_{t}")
                cand3 = cands[t][:].rearrange("p (c e) -> p c e", e=8)
                nc.vector.tensor_reduce(c8[:], cand3[:, :, 7:8],
                                        axis=mybir.AxisListType.XY,
                                        op=mybir.AluOpType.max)
                topv = smallp.tile([128, 8 * ROUNDS], f32, tag="topv",
                                   name=f"topv{t}")
                for r in range(ROUNDS):
                    nc.vector.max(topv[:, r * 8:(r + 1) * 8], cands[t][:])
                    if r < ROUNDS - 1:
                        nc.vector.match_replace(
                            cands[t][:], topv[:, r * 8:(r + 1) * 8],
                            cands[t][:], NEG)
                nc.vector.tensor_copy(taus[t][:], topv[:, k - 1:k])
                nc.vector.tensor_tensor(flags_sb[:, t:t + 1], c8[:],
                                        taus[t][:],
                                        op=mybir.AluOpType.is_gt)

            # reload/W_dec loaders; block 0 uses dedicated outer-scope pools
            # so it can prefetch during phase A
            acs = {}
            wds = {}
            pools = {"ac": None, "wd": None}        # set in the C/D scope

            def load_ac(t, blk, eng=None):
                pool = ac0p if blk == 0 else pools["ac"]
                ac = pool.tile([128, 2048], f32, tag="ac",
                               name=f"ac{t}_{blk}")
                (eng or nc.sync).dma_start(
                    ac[:],
                    acts_spill.ap()[t, :, blk * 2048:(blk + 1) * 2048])
                acs[(t, blk)] = ac

            def load_wd(blk, eng=None):
                for g in range(2):
                    pool = wd0p if blk == 0 else pools["wd"]
                    wd = pool.tile([128, 8, ACT_DIM], bf16, tag="wd",
                                   name=f"wd{blk}_{g}")
                    (eng or nc.sync).dma_start(
                        wd[:].rearrange("p c a -> p (c a)"),
                        wdecT_d.ap()[blk * 2 + g, :, :])
                    wds[(blk, g)] = wd

            # ---------------- Phase A: encode + spill + stage-1 ----------
            with tc.tile_pool(name="xt", bufs=1) as xtp, \
                 tc.tile_pool(name="cand", bufs=NT) as candp, \
                 tc.tile_pool(name="wenc", bufs=5) as wencp, \
                 tc.tile_pool(name="bounce", bufs=6) as bouncep, \
                 tc.tile_pool(name="encpsum", bufs=8, space="PSUM") as encpsp, \
                 tc.tile_pool(name="bencbc", bufs=2) as bencbcp:

                cands = [candp.tile([128, CANDW], f32, tag="cand",
                                    name=f"cand{t}") for t in range(NT)]
                x_sb = xtp.tile([128, NA, ROWS], f32r)
                nc.sync.dma_start(
                    x_sb[:], xt_d.ap().rearrange("(a p) r -> p a r", p=128))

                def load_w(sc):
                    wch = wencp.tile([128, NA, 512], f32r, tag="w",
                                     name=f"w{sc}")
                    nc.sync.dma_start(
                        wch[:],
                        wenc_d.ap()[:, sc * 512:(sc + 1) * 512]
                        .rearrange("(a p) c -> p a c", p=128))
                    if with_benc:
                        bb = bencbcp.tile([128, 512], f32, tag="bb")
                        nc.gpsimd.partition_broadcast(
                            bb[:], benc_row[0:1, sc * 512:(sc + 1) * 512])
                    else:
                        bb = None
                    return wch, bb

                def enc_chunk(sc, t, wch, bb):
                    ps = encpsp.tile([128, 512], f32, tag="eps")
                    rt = slice(t * 128, (t + 1) * 128)
                    for a in range(NA):
                        nc.tensor.matmul(
                            ps[:], x_sb[:, a, rt], wch[:, a, :],
                            start=(a == 0), stop=(a == NA - 1))
                    bo = bouncep.tile([128, 512], f32, tag="bo")
                    if with_benc:
                        nc.vector.tensor_tensor(bo[:], ps[:], bb[:],
                                                op=mybir.AluOpType.add)
                        nc.scalar.activation(
                            bo[:], bo[:], mybir.ActivationFunctionType.Relu)
                    else:
                        nc.scalar.activation(
                            bo[:], ps[:], mybir.ActivationFunctionType.Relu)
                    nc.sync.dma_start(
                        acts_spill.ap()[t, :, sc * 512:(sc + 1) * 512], bo[:])
                    for cc in range(512 // CH):
                        c = sc * (512 // CH) + cc
                        nc.vector.max(
                            cands[t][:, c * 8:(c + 1) * 8],
                            bo[:, cc * CH:(cc + 1) * CH])

                TAIL = 4
                for sc in range(NSC - TAIL):
                    if sc == NSC // 2:
                        # phase-C prefetch: blk 0 acts + first W_dec chunk
                        # (spilled long ago); dispatched from the Activation
                        # queue so the W-enc stream is undisturbed
                        for t in range(NT):
                            load_ac(t, 0, eng=nc.scalar)
                        load_wd(0, eng=nc.scalar)
                    wch, bb = load_w(sc)
                    for t in range(NT):
                        enc_chunk(sc, t, wch, bb)
                # tail: tile-major so each tile's tau extraction overlaps the
                # remaining tiles' encode matmuls on the PE
                tail_w = {sc: load_w(sc) for sc in range(NSC - TAIL, NSC)}
                for t in range(NT):
                    for sc in range(NSC - TAIL, NSC):
                        enc_chunk(sc, t, *tail_w[sc])
                    topk(t)

            # -------- Phases B+C+D: threshold, mask/transpose, decode ----
            with tc.tile_pool(name="actsc", bufs=4) as actscp, \
                 tc.tile_pool(name="encb", bufs=4) as encbp, \
                 tc.tile_pool(name="enct", bufs=9) as enctp, \
                 tc.tile_pool(name="wdec", bufs=4) as wdecp, \
                 tc.tile_pool(name="decps_hi", bufs=1, space="PSUM") as dphp, \
                 tc.tile_pool(name="decps_lo", bufs=1, space="PSUM") as dplp, \
                 tc.tile_pool(name="tpsum", bufs=2, space="PSUM") as tpp, \
                 tc.tile_pool(name="outsb", bufs=2) as outp:

                pools["ac"] = actscp
                pools["wd"] = wdecp
                ets = {}
                # decode accumulators, packed bank-aligned: [0:512] parts in
                # 4 full banks, [512:768] parts packed 2-per-bank.  The
                # packed lo chains must never issue start=True: the PSUM
                # zero-region of a start is bank-granular and would clobber
                # the neighbor chain.  memset once, accumulate start=False.
                dph = dphp.tile([128, NT, 512], f32)
                dpl = dplp.tile([128, NT, 256], f32)
                nc.vector.memset(dpl[:], 0.0)

                def mask_transpose(t, blk, ac):
                    # mask in f32 (exact selection) -> eb bf16; transpose on
                    # the PE (identity matmul) via PSUM, DVE drains to SBUF
                    eb = encbp.tile([128, 2048], bf16, tag="eb",
                                    name=f"eb{t}_{blk}")
                    nc.vector.scalar_tensor_tensor(
                        eb[:], ac[:], taus[t][:, 0:1], ac[:],
                        op0=mybir.AluOpType.is_ge,
                        op1=mybir.AluOpType.mult)
                    et = enctp.tile([128, 16, 128], bf16, tag="enct",
                                    name=f"et{t}_{blk}")
                    for h in range(2):
                        tps = tpp.tile([128, 8, 128], bf16, tag="tps")
                        for j in range(8):
                            c = h * 8 + j
                            nc.tensor.transpose(
                                tps[:, j, :],
                                eb[:, c * 128:(c + 1) * 128], ident[:])
                        nc.vector.tensor_copy(
                            et[:, h * 8:(h + 1) * 8, :], tps[:])
                    ets[(t, blk)] = et

                def decode(blk):
                    for g in range(2):
                        wd = wds.pop((blk, g))
                        for t in range(NT):
                            for j in range(8):
                                f = blk * 16 + g * 8 + j
                                lhsT = ets[(t, blk)][:, g * 8 + j, :]
                                st = (f == 0)
                                sp = (f == NF - 1)
                                nc.tensor.matmul(
                                    dph[:, t, :], lhsT, wd[:, j, 0:512],
                                    start=st, stop=sp)
                                nc.tensor.matmul(
                                    dpl[:, t, :], lhsT,
                                    wd[:, j, 512:ACT_DIM],
                                    start=False, stop=sp)
                    for t in range(NT):
                        ets.pop((t, blk))

                # taus were computed at the tail of phase A; blk 0 acts and
                # W_dec chunk 0 were prefetched mid-phase-A.  The PE can mask
                # and transpose blk 0 immediately.
                for t in range(NT):
                    load_ac(t, 1)
                    mask_transpose(t, 0, acs.pop((t, 0)))

                for blk in range(1, NBLK):
                    load_wd(blk)
                    for t in range(NT):
                        if blk + 1 < NBLK:
                            load_ac(t, blk + 1)
                        mask_transpose(t, blk, acs.pop((t, blk)))
                    decode(blk - 1)
                decode(NBLK - 1)
                for t in range(NT):
                    ot = outp.tile([128, ACT_DIM], f32, tag="ot",
                                   name=f"ot{t}")
                    nc.vector.tensor_tensor(ot[:, 0:512], dph[:, t, :],
                                            bdec_bc[:, 0:512],
                                            op=mybir.AluOpType.add)
                    nc.vector.tensor_tensor(ot[:, 512:ACT_DIM], dpl[:, t, :],
                                            bdec_bc[:, 512:ACT_DIM],
                                            op=mybir.AluOpType.add)
                    nc.sync.dma_start(
                        xhat_d.ap()[t * 128:(t + 1) * 128, :], ot[:])
                nc.sync.dma_start(flags_d.ap(), flags_sb[:])

    nc.compile()
    return nc


def _get_program(k: int, with_benc: bool):
    key = (k, with_benc)
    if key not in _cache:
        _cache[key] = _build(k, with_benc)
    return _cache[key]


def _host_repair(out, rows, x, W_enc, b_enc, W_dec, b_dec, k):
    for r in rows:
        pre = (x[r] - b_dec) @ W_enc.T + b_enc
        acts = np.maximum(pre, 0.0)
        idx = np.argsort(-acts, kind="stable")[:k]
        enc = np.zeros_like(acts)
        enc[idx] = acts[idx]
        out[r] = enc @ W_dec.T + b_dec


def run(inputs, trace=False):
    from concourse.bass_utils import run_bass_kernel_spmd

    x = np.asarray(inputs["x"], dtype=np.float32)
    W_enc = np.asarray(inputs["W_enc"], dtype=np.float32)
    b_enc = np.asarray(inputs["b_enc"], dtype=np.float32)
    W_dec = np.asarray(inputs["W_dec"], dtype=np.float32)
    b_dec = np.asarray(inputs["b_dec"], dtype=np.float32)
    k = int(np.asarray(inputs["k"]))
    assert x.shape == (BATCH, ACT_DIM) and W_enc.shape == (DICT, ACT_DIM)
    assert 1 <= k <= CANDW - 8

    with_benc = bool(np.any(b_enc))
    nc = _get_program(k, with_benc)

    xT = np.ascontiguousarray((x - b_dec).T, dtype=np.float32)
    wencT = np.ascontiguousarray(W_enc.T, dtype=np.float32)
    wdecT = np.ascontiguousarray(W_dec.T).astype(BF16)
    # [NFG, 128, 8*ACT_DIM]: partition p of group fg holds rows of the 8
    # 128-row f-chunks, giving 12KB contiguous per-partition DMA reads
    wdec_r = np.ascontiguousarray(
        wdecT.reshape(DICT // 1024, 8, 128, ACT_DIM).transpose(0, 2, 1, 3)
        .reshape(DICT // 1024, 128, 8 * ACT_DIM))
    bdec_row = np.ascontiguousarray(b_dec.reshape(1, ACT_DIM))

    in_maps = []
    for c in range(NCORES):
        sl = slice(c * ROWS, (c + 1) * ROWS)
        m = {
            "xt": np.ascontiguousarray(xT[:, sl]),
            "wenc": wencT,
            "wdecT": wdec_r,
            "bdec": bdec_row,
            "ident": np.eye(128, dtype=BF16),
        }
        if with_benc:
            m["benc"] = np.ascontiguousarray(b_enc.reshape(1, DICT))
        in_maps.append(m)

    res = run_bass_kernel_spmd(nc, in_maps, core_ids=list(range(NCORES)),
                               trace=trace)

    out = np.empty((BATCH, ACT_DIM), dtype=np.float32)
    flagged = []
    for c in range(NCORES):
        out[c * ROWS:(c + 1) * ROWS] = res.results[c]["xhat"]
        fl = res.results[c]["flags"]          # [128, NT]
        for t in range(NT):
            for p in np.nonzero(fl[:, t] > 0)[0]:
                flagged.append(c * ROWS + t * 128 + int(p))
    if flagged:
        _host_repair(out, flagged, x, W_enc, b_enc, W_dec, b_dec, k)
    return out, res, flagged


def kernel(**inputs) -> np.ndarray:
    out, _, _ = run(inputs)
    return out
